# revision 21
# baseline (speedup 1.0000x reference)
"""Block-diagonal attention kernel for Trainium2 (8 NeuronCores).

Problem: q,k,v [4, 16, 4128, 64] f32. For each (b,h): attention is computed
independently within consecutive 64-row blocks (64 full blocks) plus one
final 32-row block (4128 = 64*64 + 32).

Sharding: B*H = 64 (b,h) pairs -> 8 pairs per core (pure data parallel).

Per-core program: per (b,h) we DMA whole q/k/v heads (1 MB each) into SBUF,
then per superchunk of 512 rows (4 chunks of 128 rows = 8 blocks of 64):
  - PE pair-transposes Q,K ([128,128] -> [Qc^T;Qc+1^T] stacked on partitions)
  - per 128-chunk: one matmul S^T = K_c Q_c^T  [128,128] (cross-block
    quadrants are garbage, masked by quadrant-only exp). Even/odd chunks go
    to different PSUM banks: concurrent row-group-tiled matmuls writing the
    same PSUM bank are fatal on HW.
  - exp(S^T/8) on diagonal quadrants only -> P^T (off-diag zeroed by gpsimd)
  - per chunk: matmul P_c^T.T @ [V_c | ones] -> outputs AND row-sums
  - reciprocal of sums, per-partition scale on PSUM->SBUF copy
and one output DMA per (b,h). Softmax max-subtraction is skipped: scores
~ N(0,1), |s| < ~7, exp is safe and exact in f32.
"""
import sys

sys.path.insert(0, "/opt/trn_rl_repo")

import numpy as np
from contextlib import ExitStack

import concourse.tile as tile
from concourse import bacc, mybir
from concourse.bass_utils import run_bass_kernel_spmd
from concourse.masks import make_identity

F32 = mybir.dt.float32
AF = mybir.ActivationFunctionType

B, H, N, D = 4, 16, 4128, 64
BH_PER_CORE = 8          # 64 (b,h) pairs / 8 cores
NMAIN = 4096             # rows covered by full 64-blocks, per (b,h)
NREM = 32                # remainder block rows
SC = 512                 # superchunk rows (4 chunks of 128)
N_SC = NMAIN // SC       # 8 superchunks per (b,h)
SCALE = 1.0 / 8.0        # 1/sqrt(D)

# PSUM bank budget is 8: tqk*1 + ss*2 + o*1 banks per buf set
PSUM_BUFS = {"tqk": 2, "ss": 2, "o": 2}
REMAINDER_AFTER_BH = None  # None -> at end
PT_BUFS = 3
# Zero the pt slots once at startup instead of 2 gpsimd memsets per
# superchunk (exp only writes the diagonal quadrants, so startup zeros
# survive slot reuse). Measured neutral-to-slightly-worse vs per-superchunk
# gpsimd memsets (the Pool engine is otherwise idle), so off by default.
PT_INIT_ONCE = False
BIG_BUFS = 3
SB_BUFS = 3


def _superchunk(nc, pools, ident, qsb, ksb, vsb, outb, s):
    sb, ps = pools
    qs = qsb[:, 4 * s:4 * s + 4, :]                  # [128, 4, 64]
    ks = ksb[:, 4 * s:4 * s + 4, :]
    vs = vsb[:, 4 * s:4 * s + 4, :]                  # [128, 4, 65] w/ ones col

    # transposes: tqk cols 0:128 [Q0^T;Q1^T], 128:256 [Q2^T;Q3^T], then K
    tqk = ps.tile([128, 512], F32, tag="tqk", bufs=PSUM_BUFS["tqk"])
    qs2 = qs.rearrange("p c d -> p (c d)")
    ks2 = ks.rearrange("p c d -> p (c d)")
    nc.tensor.transpose(tqk[:, 0:128], qs2[:, 0:128], ident[:])
    nc.tensor.transpose(tqk[:, 128:256], qs2[:, 128:256], ident[:])
    nc.tensor.transpose(tqk[:, 256:384], ks2[:, 0:128], ident[:])
    nc.tensor.transpose(tqk[:, 384:512], ks2[:, 128:256], ident[:])

    tq = sb.tile([128, 512], F32, tag="tq")
    nc.vector.tensor_copy(tq[:], tqk[:])

    # S^T per chunk; chunk c -> cols 512*(c%2) + 128*(c//2) of 2-bank tile
    # (concurrent row-group-tiled matmuls must write different PSUM banks)
    ss = ps.tile([128, 1024], F32, tag="ss", bufs=PSUM_BUFS["ss"])
    for c in range(4):
        p0 = 64 * (c % 2)
        g = c // 2
        col = 512 * (c % 2) + 128 * g
        lhsT = tq[p0:p0 + 64, 256 + 128 * g:256 + 128 * g + 128]
        rhs = tq[p0:p0 + 64, 128 * g:128 * g + 128]
        nc.tensor.matmul(ss[:, col:col + 128], lhsT, rhs, tile_position=(p0, 0))

    # exp of diagonal quadrants -> pt (col groups ordered [c0|c2|c1|c3])
    pt = sb.tile([128, 512], F32, tag="pt", bufs=PT_BUFS)
    ptq = pt.rearrange("p (b g t d) -> p b g t d", b=2, g=2, t=2)
    ssq = ss.rearrange("p (b u g t d) -> p b u g t d", b=2, u=2, g=2, t=2)[:, :, 0]
    if not PT_INIT_ONCE:
        nc.gpsimd.memset(ptq[0:64, :, :, 1, :], 0.0)
        nc.gpsimd.memset(ptq[64:128, :, :, 0, :], 0.0)
    nc.scalar.activation(ptq[0:64, :, :, 0, :], ssq[0:64, :, :, 0, :], AF.Exp,
                         scale=SCALE)
    nc.scalar.activation(ptq[64:128, :, :, 1, :], ssq[64:128, :, :, 1, :], AF.Exp,
                         scale=SCALE)

    # PV per chunk: o[:, c, :] = P_c^T.T @ [V_c | 1]
    o = ps.tile([128, 4, 65], F32, tag="o", bufs=PSUM_BUFS["o"])
    for c in range(4):
        j = 2 * (c % 2) + c // 2
        nc.tensor.matmul(o[:, c, :], pt[:, 128 * j:128 * j + 128], vs[:, c, :])

    # normalize into the per-bh output tile
    r = sb.tile([128, 4], F32, tag="r")
    nc.vector.reciprocal(r[:], o[:, :, 64])
    for c in range(4):
        dst = outb[:, 4 * s + c, :]
        if c < 2:
            nc.scalar.activation(dst, o[:, c, 0:64], AF.Copy, scale=r[:, c:c + 1])
        else:
            nc.vector.tensor_scalar_mul(dst, o[:, c, 0:64], r[:, c:c + 1])


def _remainder(nc, pools, ident, qc, kc, vc, oc):
    """All 8 bh remainder blocks ([32, 64] each) in one pass on partitions
    0:32, blocks stacked along the free dim. All matmuls share row group ->
    serialized -> single-bank PSUM writes are safe."""
    sb, ps = pools

    rq = sb.tile([32, 8, 64], F32, tag="rq")
    rk = sb.tile([32, 8, 64], F32, tag="rk")
    rv = sb.tile([32, 8, 65], F32, tag="rv")
    nc.sync.dma_start(out=rq[:], in_=qc[:, NMAIN:N, :].rearrange("j p d -> p j d"))
    nc.sync.dma_start(out=rk[:], in_=kc[:, NMAIN:N, :].rearrange("j p d -> p j d"))
    nc.sync.dma_start(out=rv[:, :, 0:64],
                      in_=vc[:, NMAIN:N, :].rearrange("j p d -> p j d"))
    nc.vector.memset(rv[:, :, 64], 1.0)

    # transposes: rt[64, 0:256]=Q^T (8x[64,32]), [64, 256:512]=K^T
    rt = ps.tile([64, 512], F32, tag="tqk", bufs=PSUM_BUFS["tqk"])
    for j in range(8):
        nc.tensor.transpose(rt[:, 32 * j:32 * j + 32], rq[:, j, :],
                            ident[0:32, 0:32])
        nc.tensor.transpose(rt[:, 256 + 32 * j:256 + 32 * j + 32], rk[:, j, :],
                            ident[0:32, 0:32])
    rts = sb.tile([64, 512], F32, tag="rts")
    nc.vector.tensor_copy(rts[:], rt[:])

    # S^T per block: [32, 32] at partitions 0:32, serialized, one bank
    rss = ps.tile([32, 8, 32], F32, tag="o", bufs=PSUM_BUFS["o"])
    for j in range(8):
        nc.tensor.matmul(rss[:, j, :], rts[:, 256 + 32 * j:256 + 32 * j + 32],
                         rts[:, 32 * j:32 * j + 32])

    rpt = sb.tile([32, 8, 32], F32, tag="rpt")
    nc.scalar.activation(rpt[:], rss[:], AF.Exp, scale=SCALE)

    # PV per block: [32, 65] at cols 128j of a 2-bank tile (no crossing)
    ro = ps.tile([32, 8, 128], F32, tag="ss", bufs=PSUM_BUFS["ss"])
    for j in range(8):
        nc.tensor.matmul(ro[:, j, 0:65], rpt[:, j, :], rv[:, j, :])

    rr = sb.tile([32, 8], F32, tag="rr")
    nc.vector.reciprocal(rr[:], ro[:, :, 64])
    routs = sb.tile([32, 8, 64], F32, tag="routs")
    for j in range(8):
        if j % 2 == 0:
            nc.scalar.activation(routs[:, j, :], ro[:, j, 0:64], AF.Copy,
                                 scale=rr[:, j:j + 1])
        else:
            nc.vector.tensor_scalar_mul(routs[:, j, :], ro[:, j, 0:64],
                                        rr[:, j:j + 1])

    nc.sync.dma_start(out=oc[:, NMAIN:N, :].rearrange("j p d -> p j d"),
                      in_=routs[:])


def build_nc(repeat=1):
    nc = bacc.Bacc("TRN2", target_bir_lowering=False, debug=False, num_devices=8)
    qc = nc.dram_tensor("qc", [BH_PER_CORE, N, D], F32, kind="ExternalInput").ap()
    kc = nc.dram_tensor("kc", [BH_PER_CORE, N, D], F32, kind="ExternalInput").ap()
    vc = nc.dram_tensor("vc", [BH_PER_CORE, N, D], F32, kind="ExternalInput").ap()
    oc = nc.dram_tensor("oc", [BH_PER_CORE, N, D], F32, kind="ExternalOutput").ap()

    with tile.TileContext(nc) as tc, ExitStack() as ctx:
        singles = ctx.enter_context(tc.tile_pool(name="singles", bufs=1))
        big = ctx.enter_context(tc.tile_pool(name="big", bufs=BIG_BUFS))
        sb = ctx.enter_context(tc.tile_pool(name="sb", bufs=SB_BUFS))
        ps = ctx.enter_context(tc.tile_pool(name="ps", bufs=2, space="PSUM"))
        pools = (sb, ps)

        ident = singles.tile([128, 128], F32)
        make_identity(nc, ident[:])

        if PT_INIT_ONCE:
            # pre-zero every pt slot; the loop's pt tiles reuse these slots
            # round-robin and exp never writes the off-diagonal quadrants
            for _ in range(PT_BUFS):
                pt0 = sb.tile([128, 512], F32, tag="pt", bufs=PT_BUFS)
                nc.gpsimd.memset(pt0[:], 0.0)

        for _ in range(repeat):
            for bh in range(BH_PER_CORE):
                # whole-head loads: 1 MB per DMA, spread across the three
                # DGE issuers (SP-HWDGE, ACT-HWDGE, Pool-SWDGE) — measured
                # 162 -> 135 us for the pure-DMA pattern
                qsb = big.tile([128, 32, 64], F32, tag="qsb")
                ksb = big.tile([128, 32, 64], F32, tag="ksb")
                vsb = big.tile([128, 32, 65], F32, tag="vsb")
                nc.sync.dma_start(
                    out=qsb[:],
                    in_=qc[bh, 0:NMAIN, :].rearrange("(ci p) d -> p ci d", p=128))
                nc.scalar.dma_start(
                    out=ksb[:],
                    in_=kc[bh, 0:NMAIN, :].rearrange("(ci p) d -> p ci d", p=128))
                nc.gpsimd.dma_start(
                    out=vsb[:, :, 0:64],
                    in_=vc[bh, 0:NMAIN, :].rearrange("(ci p) d -> p ci d", p=128))
                nc.vector.memset(vsb[:, :, 64], 1.0)
                outb = big.tile([128, 32, 64], F32, tag="outb")
                for s in range(N_SC):
                    _superchunk(nc, pools, ident, qsb, ksb, vsb, outb, s)
                out_eng = nc.sync if bh % 2 == 0 else nc.scalar
                out_eng.dma_start(
                    out=oc[bh, 0:NMAIN, :].rearrange("(ci p) d -> p ci d", p=128),
                    in_=outb[:])
                if REMAINDER_AFTER_BH == bh:
                    _remainder(nc, pools, ident, qc, kc, vc, oc)
            if REMAINDER_AFTER_BH is None:
                _remainder(nc, pools, ident, qc, kc, vc, oc)

    nc.compile()
    return nc


_CACHE = {}


def kernel(q, k, v):
    assert q.shape == (B, H, N, D), q.shape
    if "nc" not in _CACHE:
        _CACHE["nc"] = build_nc()
    nc = _CACHE["nc"]

    q64 = np.ascontiguousarray(np.asarray(q, dtype=np.float32).reshape(B * H, N, D))
    k64 = np.ascontiguousarray(np.asarray(k, dtype=np.float32).reshape(B * H, N, D))
    v64 = np.ascontiguousarray(np.asarray(v, dtype=np.float32).reshape(B * H, N, D))

    in_maps = []
    for i in range(8):
        sl = slice(BH_PER_CORE * i, BH_PER_CORE * (i + 1))
        in_maps.append({"qc": q64[sl], "kc": k64[sl], "vc": v64[sl]})

    # One retry: rapid repeated executions occasionally wedge a core with a
    # transient NRT_EXEC_UNIT_UNRECOVERABLE; a fresh attempt recovers.
    try:
        res = run_bass_kernel_spmd(nc, in_maps, core_ids=list(range(8)))
    except Exception:
        import time
        time.sleep(2.0)
        res = run_bass_kernel_spmd(nc, in_maps, core_ids=list(range(8)))
    out = np.empty((B * H, N, D), dtype=np.float32)
    for i in range(8):
        out[BH_PER_CORE * i:BH_PER_CORE * (i + 1)] = res.results[i]["oc"]
    return out.reshape(B, H, N, D)
